# Initial kernel scaffold
#
"""AGNN layer (cosine-attention message passing) on 8 TRN2 NeuronCores.

Strategy: edges are sorted by destination node on the host; the node range is
cut into blocks of <= 128 nodes and <= TPB*128 edges; contiguous blocks go to
each of the 8 cores (edge/data parallel with node-aligned cuts), so every
softmax segment lives entirely on one core and no collectives are needed.

Device kernel per core:
  prologue: cast x (f32) -> bf16 table (src rows / messages) and fp8-e4m3
            table (dst rows, only used inside the cosine logit).
  per 128-edge tile: indirect-DMA gather of src rows (bf16) and dst rows
            (fp8); per-edge cosine via fused multiply+reduce; w = exp(beta *
            cos); one-hot(dst)*w matrix via iota/is_equal; two PSUM-
            accumulating matmuls produce per-node  sum(w * x_src)  and
            sum(w)  for the block.
  epilogue per block: out_rows = relu(M / s), indirect-scatter to out.

Logits are cosines scaled by beta, bounded, so exp() never overflows and the
reference's segment-max subtraction cancels exactly -- single pass suffices.
"""

import numpy as np
import ml_dtypes

import concourse.bacc as bacc
import concourse.bass as bass
import concourse.mybir as mybir
import concourse.tile as tile
from concourse.bass import IndirectOffsetOnAxis

P = 128
N_NODES = 50000
D_FEAT = 128
N_EDGES = 1600000
NCORES = 8
TPB = 34  # tiles (of 128 edges) per block; block edge capacity = TPB*128
NODE_SENTINEL = 1 << 30
BLK_SENTINEL = 300.0

F32 = mybir.dt.float32
BF16 = mybir.dt.bfloat16
F8 = mybir.dt.float8e4
I32 = mybir.dt.int32
I16 = mybir.dt.int16

BF16_NP = ml_dtypes.bfloat16
F8_NP = ml_dtypes.float8_e4m3fn


def _build_graph(N, D, T, NB, tpb):
    """One SPMD graph, identical on all cores; per-core data differs."""
    nc = bacc.Bacc(
        "TRN2", target_bir_lowering=False, debug=False, enable_asserts=False
    )
    x_ext = nc.dram_tensor("x", [N, D], F32, kind="ExternalInput").ap()
    src_idx = nc.dram_tensor("src_idx", [P, T], I32, kind="ExternalInput").ap()
    dst_idx = nc.dram_tensor("dst_idx", [P, T], I32, kind="ExternalInput").ap()
    dst_blk = nc.dram_tensor("dst_blk", [P, T], BF16, kind="ExternalInput").ap()
    node_ids = nc.dram_tensor("node_ids", [P, NB], I32, kind="ExternalInput").ap()
    beta128 = nc.dram_tensor("beta128", [P, 1], F32, kind="ExternalInput").ap()
    out_ext = nc.dram_tensor("out", [N, D], F32, kind="ExternalOutput").ap()

    xb_tab = nc.dram_tensor("xb_table", [N, D], BF16).ap()
    x8_tab = nc.dram_tensor("x8_table", [N, D], F8).ap()

    with tile.TileContext(nc) as tc:
        with (
            tc.tile_pool(name="const", bufs=1) as constp,
            tc.tile_pool(name="prolog", bufs=3) as prologp,
            tc.tile_pool(name="idx", bufs=2) as idxp,
            tc.tile_pool(name="gsrc", bufs=2) as gsrcp,
            tc.tile_pool(name="gdst", bufs=2) as gdstp,
            tc.tile_pool(name="work", bufs=3) as workp,
            tc.tile_pool(name="cols", bufs=4) as colp,
            tc.tile_pool(name="orow", bufs=2) as orowp,
            tc.tile_pool(name="psum", bufs=2, space="PSUM") as psump,
        ):
            # ---- constants ----
            iota_i16 = constp.tile([P, P], I16)
            nc.gpsimd.iota(iota_i16[:], pattern=[[1, P]], base=0, channel_multiplier=0)
            iota_bf = constp.tile([P, P], BF16)
            nc.vector.tensor_copy(iota_bf[:], iota_i16[:])
            ones_bf = constp.tile([P, 1], BF16)
            nc.vector.memset(ones_bf[:], 1.0)
            beta_sb = constp.tile([P, 1], F32)
            nc.sync.dma_start(out=beta_sb[:], in_=beta128[:, :])

            # ---- prologue: cast x -> bf16 and fp8 tables ----
            ROWS_PP = 8  # rows of x per partition per supertile
            SUPER = P * ROWS_PP  # 1024 rows
            r0 = 0
            while r0 < N:
                rows = min(SUPER, N - r0)
                pp = rows // ROWS_PP
                assert pp * ROWS_PP == rows, (r0, rows)
                xt = prologp.tile([P, ROWS_PP, D], F32, tag="xt")
                nc.sync.dma_start(out=xt[:pp], in_=x_ext[r0 : r0 + rows, :])
                xbt = prologp.tile([P, ROWS_PP, D], BF16, tag="xbt")
                nc.vector.tensor_copy(xbt[:pp], xt[:pp])
                x8t = prologp.tile([P, ROWS_PP, D], F8, tag="x8t")
                nc.vector.tensor_copy(x8t[:pp], xt[:pp])
                nc.scalar.dma_start(out=xb_tab[r0 : r0 + rows, :], in_=xbt[:pp])
                nc.scalar.dma_start(out=x8_tab[r0 : r0 + rows, :], in_=x8t[:pp])
                r0 += rows

            # ---- edge loop ----
            for b in range(NB):
                c0 = b * tpb
                sidx = idxp.tile([P, tpb], I32, tag="sidx")
                nc.sync.dma_start(out=sidx[:], in_=src_idx[:, c0 : c0 + tpb])
                didx = idxp.tile([P, tpb], I32, tag="didx")
                nc.sync.dma_start(out=didx[:], in_=dst_idx[:, c0 : c0 + tpb])
                dblk = idxp.tile([P, tpb], BF16, tag="dblk")
                nc.sync.dma_start(out=dblk[:], in_=dst_blk[:, c0 : c0 + tpb])
                nid = idxp.tile([P, 1], I32, tag="nid")
                nc.sync.dma_start(out=nid[:], in_=node_ids[:, b : b + 1])

                xs_blk = gsrcp.tile([P, tpb * D], BF16, tag="xs")
                nc.gpsimd.indirect_dma_start(
                    out=xs_blk[:],
                    out_offset=None,
                    in_=xb_tab[:, :],
                    in_offset=IndirectOffsetOnAxis(ap=sidx[:], axis=0),
                )
                xd_blk = gdstp.tile([P, tpb * D], F8, tag="xd8")
                nc.gpsimd.indirect_dma_start(
                    out=xd_blk[:],
                    out_offset=None,
                    in_=x8_tab[:, :],
                    in_offset=IndirectOffsetOnAxis(ap=didx[:], axis=0),
                )

                psum = psump.tile([P, D + 1], F32, tag="acc")

                for t in range(tpb):
                    xs = xs_blk[:, t * D : (t + 1) * D]
                    xd8 = xd_blk[:, t * D : (t + 1) * D]
                    xd = workp.tile([P, D], BF16, tag="xd")
                    nc.vector.tensor_copy(xd[:], xd8)

                    # ss = sum(xs^2) on ACT; dd = sum(xd^2), dot = sum(xs*xd) on DVE
                    sq = workp.tile([P, D], BF16, tag="sq")
                    ss = colp.tile([P, 1], F32, tag="ss")
                    nc.scalar.activation(
                        out=sq[:], in_=xs, func=mybir.ActivationFunctionType.Square,
                        accum_out=ss[:],
                    )
                    j1 = workp.tile([P, D], BF16, tag="j1")
                    dd = colp.tile([P, 1], F32, tag="dd")
                    nc.vector.tensor_tensor_reduce(
                        out=j1[:], in0=xd[:], in1=xd[:], scale=1.0, scalar=0.0,
                        op0=mybir.AluOpType.mult, op1=mybir.AluOpType.add,
                        accum_out=dd[:],
                    )
                    j2 = workp.tile([P, D], BF16, tag="j2")
                    dt = colp.tile([P, 1], F32, tag="dt")
                    nc.vector.tensor_tensor_reduce(
                        out=j2[:], in0=xs, in1=xd[:], scale=1.0, scalar=0.0,
                        op0=mybir.AluOpType.mult, op1=mybir.AluOpType.add,
                        accum_out=dt[:],
                    )

                    nrm2 = colp.tile([P, 1], F32, tag="nrm2")
                    nc.vector.tensor_tensor(
                        out=nrm2[:], in0=ss[:], in1=dd[:], op=mybir.AluOpType.mult
                    )
                    nrm = colp.tile([P, 1], F32, tag="nrm")
                    nc.scalar.activation(
                        out=nrm[:], in_=nrm2[:], func=mybir.ActivationFunctionType.Sqrt
                    )
                    rs = colp.tile([P, 1], F32, tag="rs")
                    nc.vector.reciprocal(rs[:], nrm[:])
                    rsb = colp.tile([P, 1], F32, tag="rsb")
                    nc.vector.tensor_tensor(
                        out=rsb[:], in0=rs[:], in1=beta_sb[:], op=mybir.AluOpType.mult
                    )
                    w = colp.tile([P, 1], F32, tag="w")
                    nc.scalar.activation(
                        out=w[:], in_=dt[:], func=mybir.ActivationFunctionType.Exp,
                        scale=rsb[:],
                    )
                    wb = colp.tile([P, 1], BF16, tag="wb")
                    nc.vector.tensor_copy(wb[:], w[:])

                    pw = workp.tile([P, P], BF16, tag="pw")
                    nc.vector.tensor_scalar(
                        out=pw[:], in0=iota_bf[:],
                        scalar1=dblk[:, t : t + 1], scalar2=wb[:],
                        op0=mybir.AluOpType.is_equal, op1=mybir.AluOpType.mult,
                    )

                    nc.tensor.matmul(
                        out=psum[:, 0:D], lhsT=pw[:], rhs=xs,
                        start=(t == 0), stop=(t == tpb - 1),
                    )
                    nc.tensor.matmul(
                        out=psum[:, D : D + 1], lhsT=pw[:], rhs=ones_bf[:],
                        start=(t == 0), stop=(t == tpb - 1),
                    )

                # epilogue: rows = relu(M / s); scatter to out
                s_safe = colp.tile([P, 1], F32, tag="ssafe")
                nc.vector.tensor_scalar(
                    out=s_safe[:], in0=psum[:, D : D + 1], scalar1=1e-30,
                    scalar2=None, op0=mybir.AluOpType.max,
                )
                sinv = colp.tile([P, 1], F32, tag="sinv")
                nc.vector.reciprocal(sinv[:], s_safe[:])
                orow = orowp.tile([P, D], F32, tag="orow")
                nc.vector.tensor_scalar(
                    out=orow[:], in0=psum[:, 0:D], scalar1=sinv[:], scalar2=0.0,
                    op0=mybir.AluOpType.mult, op1=mybir.AluOpType.max,
                )
                nc.gpsimd.indirect_dma_start(
                    out=out_ext[:, :],
                    out_offset=IndirectOffsetOnAxis(ap=nid[:], axis=0),
                    in_=orow[:],
                    in_offset=None,
                    bounds_check=N - 1,
                    oob_is_err=False,
                )

    nc.compile()
    return nc


def _decompose(dst_sorted, N, cap_edges, max_nodes=P):
    """Cut sorted-by-dst edges into blocks of <=max_nodes consecutive nodes
    and <=cap_edges edges. Returns list of (n0, n1, e0, e1)."""
    deg = np.bincount(dst_sorted, minlength=N)
    assert deg.max() <= cap_edges, "single node exceeds block capacity"
    blocks = []
    n0 = 0
    e0 = 0
    ecount = 0
    for node in range(N):
        d = int(deg[node])
        if (node - n0) >= max_nodes or ecount + d > cap_edges:
            blocks.append((n0, node, e0, e0 + ecount))
            n0 = node
            e0 = e0 + ecount
            ecount = 0
        ecount += d
    blocks.append((n0, N, e0, e0 + ecount))
    return blocks


def _prep_inputs(x, beta, edge_index, N, D, tpb, ncores):
    """Host-side sharding: self-loops, sort by dst, block decomposition,
    per-core [128, T] edge arrays."""
    E = edge_index.shape[1]
    loop = np.arange(N, dtype=np.int64)
    src = np.concatenate([np.asarray(edge_index[0]), loop]).astype(np.int64)
    dst = np.concatenate([np.asarray(edge_index[1]), loop]).astype(np.int64)
    order = np.argsort(dst, kind="stable")
    src_s = src[order].astype(np.int32)
    dst_s = dst[order].astype(np.int32)

    cap = tpb * P
    blocks = _decompose(dst_s, N, cap)
    nbt = len(blocks)
    # near-equal contiguous split of blocks across cores
    sizes = [nbt // ncores + (1 if i < nbt % ncores else 0) for i in range(ncores)]
    NB = max(sizes)
    T = NB * tpb

    xf32 = np.ascontiguousarray(np.asarray(x), dtype=np.float32)
    beta128 = np.full((P, 1), float(np.asarray(beta).reshape(-1)[0]), np.float32)

    in_maps = []
    core_ranges = []
    bpos = 0
    for k in range(ncores):
        nb_k = sizes[k]
        blks = blocks[bpos : bpos + nb_k]
        bpos += nb_k
        a_src = np.zeros((NB * tpb, P), np.int32)
        a_dst = np.zeros((NB * tpb, P), np.int32)
        a_blk = np.full((NB * tpb, P), BLK_SENTINEL, np.float32)
        a_nid = np.full((P, NB), NODE_SENTINEL, np.int32)
        for bi, (n0, n1, e0, e1) in enumerate(blks):
            ne = e1 - e0
            assert ne <= cap and (n1 - n0) <= P
            es = np.zeros(cap, np.int32)
            ed = np.zeros(cap, np.int32)
            eb = np.full(cap, BLK_SENTINEL, np.float32)
            es[:ne] = src_s[e0:e1]
            ed[:ne] = dst_s[e0:e1]
            eb[:ne] = (dst_s[e0:e1] - n0).astype(np.float32)
            a_src[bi * tpb : (bi + 1) * tpb] = es.reshape(tpb, P)
            a_dst[bi * tpb : (bi + 1) * tpb] = ed.reshape(tpb, P)
            a_blk[bi * tpb : (bi + 1) * tpb] = eb.reshape(tpb, P)
            ids = n0 + np.arange(P, dtype=np.int32)
            ids[n1 - n0 :] = NODE_SENTINEL
            a_nid[:, bi] = ids
        in_maps.append(
            {
                "x": xf32,
                "src_idx": np.ascontiguousarray(a_src.T),
                "dst_idx": np.ascontiguousarray(a_dst.T),
                "dst_blk": np.ascontiguousarray(a_blk.T).astype(BF16_NP),
                "node_ids": a_nid,
                "beta128": beta128,
            }
        )
        if blks:
            core_ranges.append((blks[0][0], blks[-1][1]))
        else:
            core_ranges.append((0, 0))
    return in_maps, core_ranges, NB, T


def _run(x, beta, edge_index, trace=False):
    from concourse.bass_utils import run_bass_kernel_spmd

    N, D = x.shape
    in_maps, core_ranges, NB, T = _prep_inputs(
        x, beta, edge_index, N, D, TPB, NCORES
    )
    nc = _build_graph(N, D, T, NB, TPB)
    res = run_bass_kernel_spmd(
        nc, in_maps, core_ids=list(range(NCORES)), trace=trace
    )
    out = np.zeros((N, D), np.float32)
    for k, (lo, hi) in enumerate(core_ranges):
        if hi > lo:
            out[lo:hi] = res.results[k]["out"][lo:hi]
    return out, res


def kernel(x, beta, edge_index):
    out, _ = _run(
        np.asarray(x), np.asarray(beta), np.asarray(edge_index), trace=False
    )
    return out


# revision 6
# speedup vs baseline: 1.1715x; 1.1715x over previous
"""AGNN layer (cosine-attention message passing) on 8 TRN2 NeuronCores.

Strategy: edges are sorted by destination node on the host; the node range is
cut into blocks of <= 128 nodes and <= TPB*128 edges; contiguous blocks go to
each of the 8 cores (edge/data parallel with node-aligned cuts), so every
softmax segment lives entirely on one core and no collectives are needed.

Device kernel per core:
  prologue: cast x (f32) -> bf16 table (src rows / messages) and fp8-e4m3
            table (dst rows, only used inside the cosine logit).
  per 128-edge tile: indirect-DMA gather of src rows (bf16) and dst rows
            (fp8); per-edge cosine via fused multiply+reduce; w = exp(beta *
            cos); one-hot(dst)*w matrix via iota/is_equal; two PSUM-
            accumulating matmuls produce per-node  sum(w * x_src)  and
            sum(w)  for the block.
  epilogue per block: out_rows = relu(M / s), indirect-scatter to out.

Logits are cosines scaled by beta, bounded, so exp() never overflows and the
reference's segment-max subtraction cancels exactly -- single pass suffices.
"""

import numpy as np
import ml_dtypes

import concourse.bacc as bacc
import concourse.bass as bass
import concourse.mybir as mybir
import concourse.tile as tile
from concourse.bass import IndirectOffsetOnAxis

P = 128
N_NODES = 50000
D_FEAT = 128
N_EDGES = 1600000
NCORES = 8
TPB = 34  # tiles (of 128 edges) per block; block edge capacity = TPB*128
NODE_SENTINEL = 8_000_000  # oob for bounds_check, no int32 overflow when *D
BLK_SENTINEL = 300.0

F32 = mybir.dt.float32
BF16 = mybir.dt.bfloat16
F8 = mybir.dt.float8e4
I32 = mybir.dt.int32
I16 = mybir.dt.int16

BF16_NP = ml_dtypes.bfloat16
F8_NP = ml_dtypes.float8_e4m3fn


def _build_graph(N, D, T, NB, tpb):
    """One SPMD graph, identical on all cores; per-core data differs."""
    nc = bacc.Bacc(
        "TRN2", target_bir_lowering=False, debug=False, enable_asserts=False
    )
    x_ext = nc.dram_tensor("x", [N, D], F32, kind="ExternalInput").ap()
    src_idx = nc.dram_tensor("src_idx", [P, T], I32, kind="ExternalInput").ap()
    dst_idx = nc.dram_tensor("dst_idx", [P, T], I32, kind="ExternalInput").ap()
    dst_blk = nc.dram_tensor("dst_blk", [P, T], F32, kind="ExternalInput").ap()
    node_ids = nc.dram_tensor("node_ids", [P, NB], I32, kind="ExternalInput").ap()
    beta128 = nc.dram_tensor("beta128", [P, 1], F32, kind="ExternalInput").ap()
    out_ext = nc.dram_tensor("out", [N, D], F32, kind="ExternalOutput").ap()

    xb_tab = nc.dram_tensor("xb_table", [N, D], BF16).ap()
    x8_tab = nc.dram_tensor("x8_table", [N, D], F8).ap()

    with tile.TileContext(nc) as tc:
        with (
            tc.tile_pool(name="const", bufs=1) as constp,
            tc.tile_pool(name="prolog", bufs=3) as prologp,
            tc.tile_pool(name="idx", bufs=2) as idxp,
            tc.tile_pool(name="gsrc", bufs=2) as gsrcp,
            tc.tile_pool(name="gdst", bufs=2) as gdstp,
            tc.tile_pool(name="work", bufs=3) as workp,
            tc.tile_pool(name="cols", bufs=4) as colp,
            tc.tile_pool(name="orow", bufs=2) as orowp,
            tc.tile_pool(name="psum", bufs=2, space="PSUM") as psump,
        ):
            # ---- constants ----
            iota_i16 = constp.tile([P, P], I16)
            nc.gpsimd.iota(iota_i16[:], pattern=[[1, P]], base=0, channel_multiplier=0)
            iota_bf = constp.tile([P, P], BF16)
            nc.vector.tensor_copy(iota_bf[:], iota_i16[:])
            ones_bf = constp.tile([P, 1], BF16)
            nc.vector.memset(ones_bf[:], 1.0)
            beta_sb = constp.tile([P, 1], F32)
            nc.sync.dma_start(out=beta_sb[:], in_=beta128[:, :])

            # ---- prologue: cast x -> bf16 and fp8 tables ----
            ROWS_PP = 8  # rows of x per partition per supertile
            SUPER = P * ROWS_PP  # 1024 rows
            r0 = 0
            while r0 < N:
                rows = min(SUPER, N - r0)
                pp = rows // ROWS_PP
                assert pp * ROWS_PP == rows, (r0, rows)
                xt = prologp.tile([P, ROWS_PP, D], F32, tag="xt")
                nc.sync.dma_start(out=xt[:pp], in_=x_ext[r0 : r0 + rows, :])
                xbt = prologp.tile([P, ROWS_PP, D], BF16, tag="xbt")
                nc.vector.tensor_copy(xbt[:pp], xt[:pp])
                x8t = prologp.tile([P, ROWS_PP, D], F8, tag="x8t")
                nc.vector.tensor_copy(x8t[:pp], xt[:pp])
                nc.scalar.dma_start(out=xb_tab[r0 : r0 + rows, :], in_=xbt[:pp])
                nc.scalar.dma_start(out=x8_tab[r0 : r0 + rows, :], in_=x8t[:pp])
                r0 += rows

            # ---- edge loop ----
            for b in range(NB):
                c0 = b * tpb
                sidx = idxp.tile([P, tpb], I32, tag="sidx")
                nc.sync.dma_start(out=sidx[:], in_=src_idx[:, c0 : c0 + tpb])
                didx = idxp.tile([P, tpb], I32, tag="didx")
                nc.sync.dma_start(out=didx[:], in_=dst_idx[:, c0 : c0 + tpb])
                dblk = idxp.tile([P, tpb], F32, tag="dblk")
                nc.sync.dma_start(out=dblk[:], in_=dst_blk[:, c0 : c0 + tpb])
                nid = idxp.tile([P, 1], I32, tag="nid")
                nc.sync.dma_start(out=nid[:], in_=node_ids[:, b : b + 1])

                xs_blk = gsrcp.tile([P, tpb * D], BF16, tag="xs")
                nc.gpsimd.indirect_dma_start(
                    out=xs_blk[:],
                    out_offset=None,
                    in_=xb_tab[:, :],
                    in_offset=IndirectOffsetOnAxis(ap=sidx[:], axis=0),
                )
                xd_blk = gdstp.tile([P, tpb * D], F8, tag="xd8")
                nc.gpsimd.indirect_dma_start(
                    out=xd_blk[:],
                    out_offset=None,
                    in_=x8_tab[:, :],
                    in_offset=IndirectOffsetOnAxis(ap=didx[:], axis=0),
                )

                psum = psump.tile([P, D], F32, tag="acc")
                psum_s = psump.tile([P, 1], F32, tag="accs")

                for t in range(tpb):
                    xs = xs_blk[:, t * D : (t + 1) * D]
                    xd8 = xd_blk[:, t * D : (t + 1) * D]
                    xd = workp.tile([P, D], BF16, tag="xd")
                    nc.vector.tensor_copy(xd[:], xd8)

                    # ss = sum(xs^2) on ACT; dd = sum(xd^2), dot = sum(xs*xd) on DVE
                    sq = workp.tile([P, D], BF16, tag="sq")
                    ss = colp.tile([P, 1], F32, tag="ss")
                    nc.scalar.activation(
                        out=sq[:], in_=xs, func=mybir.ActivationFunctionType.Square,
                        accum_out=ss[:],
                    )
                    j1 = workp.tile([P, D], BF16, tag="j1")
                    dd = colp.tile([P, 1], F32, tag="dd")
                    nc.vector.tensor_tensor(
                        out=j1[:], in0=xd[:], in1=xd[:], op=mybir.AluOpType.mult
                    )
                    nc.vector.tensor_reduce(
                        out=dd[:], in_=j1[:], axis=mybir.AxisListType.X,
                        op=mybir.AluOpType.add,
                    )
                    j2 = workp.tile([P, D], BF16, tag="j2")
                    dt = colp.tile([P, 1], F32, tag="dt")
                    nc.vector.tensor_tensor(
                        out=j2[:], in0=xs, in1=xd[:], op=mybir.AluOpType.mult
                    )
                    nc.vector.tensor_reduce(
                        out=dt[:], in_=j2[:], axis=mybir.AxisListType.X,
                        op=mybir.AluOpType.add,
                    )

                    nrm2 = colp.tile([P, 1], F32, tag="nrm2")
                    nc.vector.tensor_tensor(
                        out=nrm2[:], in0=ss[:], in1=dd[:], op=mybir.AluOpType.mult
                    )
                    nrm = colp.tile([P, 1], F32, tag="nrm")
                    nc.scalar.activation(
                        out=nrm[:], in_=nrm2[:], func=mybir.ActivationFunctionType.Sqrt
                    )
                    rs = colp.tile([P, 1], F32, tag="rs")
                    nc.vector.reciprocal(rs[:], nrm[:])
                    rsb = colp.tile([P, 1], F32, tag="rsb")
                    nc.vector.tensor_tensor(
                        out=rsb[:], in0=rs[:], in1=beta_sb[:], op=mybir.AluOpType.mult
                    )
                    w = colp.tile([P, 1], F32, tag="w")
                    nc.scalar.activation(
                        out=w[:], in_=dt[:], func=mybir.ActivationFunctionType.Exp,
                        scale=rsb[:],
                    )
                    pw = workp.tile([P, P], BF16, tag="pw")
                    nc.vector.tensor_scalar(
                        out=pw[:], in0=iota_bf[:],
                        scalar1=dblk[:, t : t + 1], scalar2=w[:],
                        op0=mybir.AluOpType.is_equal, op1=mybir.AluOpType.mult,
                    )

                    nc.tensor.matmul(
                        out=psum[:, 0:D], lhsT=pw[:], rhs=xs,
                        start=(t == 0), stop=(t == tpb - 1),
                    )
                    nc.tensor.matmul(
                        out=psum_s[:, 0:1], lhsT=pw[:], rhs=ones_bf[:],
                        start=(t == 0), stop=(t == tpb - 1),
                    )

                # epilogue: rows = relu(M / s); scatter to out
                s_safe = colp.tile([P, 1], F32, tag="ssafe")
                nc.vector.tensor_scalar(
                    out=s_safe[:], in0=psum_s[:, 0:1], scalar1=1e-30,
                    scalar2=None, op0=mybir.AluOpType.max,
                )
                sinv = colp.tile([P, 1], F32, tag="sinv")
                nc.vector.reciprocal(sinv[:], s_safe[:])
                orow = orowp.tile([P, D], F32, tag="orow")
                nc.vector.tensor_scalar(
                    out=orow[:], in0=psum[:, 0:D], scalar1=sinv[:], scalar2=0.0,
                    op0=mybir.AluOpType.mult, op1=mybir.AluOpType.max,
                )
                nc.gpsimd.indirect_dma_start(
                    out=out_ext[:, :],
                    out_offset=IndirectOffsetOnAxis(ap=nid[:], axis=0),
                    in_=orow[:],
                    in_offset=None,
                    bounds_check=N - 1,
                    oob_is_err=False,
                )

    nc.compile()
    return nc


def _decompose(dst_sorted, N, cap_edges, max_nodes=P):
    """Cut sorted-by-dst edges into blocks of <=max_nodes consecutive nodes
    and <=cap_edges edges. Returns list of (n0, n1, e0, e1)."""
    deg = np.bincount(dst_sorted, minlength=N)
    assert deg.max() <= cap_edges, "single node exceeds block capacity"
    blocks = []
    n0 = 0
    e0 = 0
    ecount = 0
    for node in range(N):
        d = int(deg[node])
        if (node - n0) >= max_nodes or ecount + d > cap_edges:
            blocks.append((n0, node, e0, e0 + ecount))
            n0 = node
            e0 = e0 + ecount
            ecount = 0
        ecount += d
    blocks.append((n0, N, e0, e0 + ecount))
    return blocks


def _prep_inputs(x, beta, edge_index, N, D, tpb, ncores):
    """Host-side sharding: self-loops, sort by dst, block decomposition,
    per-core [128, T] edge arrays."""
    E = edge_index.shape[1]
    loop = np.arange(N, dtype=np.int64)
    src = np.concatenate([np.asarray(edge_index[0]), loop]).astype(np.int64)
    dst = np.concatenate([np.asarray(edge_index[1]), loop]).astype(np.int64)
    order = np.argsort(dst, kind="stable")
    src_s = src[order].astype(np.int32)
    dst_s = dst[order].astype(np.int32)

    cap = tpb * P
    blocks = _decompose(dst_s, N, cap)
    nbt = len(blocks)
    # near-equal contiguous split of blocks across cores
    sizes = [nbt // ncores + (1 if i < nbt % ncores else 0) for i in range(ncores)]
    NB = max(sizes)
    T = NB * tpb

    xf32 = np.ascontiguousarray(np.asarray(x), dtype=np.float32)
    beta128 = np.full((P, 1), float(np.asarray(beta).reshape(-1)[0]), np.float32)

    in_maps = []
    core_ranges = []
    bpos = 0
    for k in range(ncores):
        nb_k = sizes[k]
        blks = blocks[bpos : bpos + nb_k]
        bpos += nb_k
        a_src = np.zeros((NB * tpb, P), np.int32)
        a_dst = np.zeros((NB * tpb, P), np.int32)
        a_blk = np.full((NB * tpb, P), BLK_SENTINEL, np.float32)
        a_nid = np.full((P, NB), NODE_SENTINEL, np.int32)
        for bi, (n0, n1, e0, e1) in enumerate(blks):
            ne = e1 - e0
            assert ne <= cap and (n1 - n0) <= P
            es = np.zeros(cap, np.int32)
            ed = np.zeros(cap, np.int32)
            eb = np.full(cap, BLK_SENTINEL, np.float32)
            es[:ne] = src_s[e0:e1]
            ed[:ne] = dst_s[e0:e1]
            eb[:ne] = (dst_s[e0:e1] - n0).astype(np.float32)
            a_src[bi * tpb : (bi + 1) * tpb] = es.reshape(tpb, P)
            a_dst[bi * tpb : (bi + 1) * tpb] = ed.reshape(tpb, P)
            a_blk[bi * tpb : (bi + 1) * tpb] = eb.reshape(tpb, P)
            ids = n0 + np.arange(P, dtype=np.int32)
            ids[n1 - n0 :] = NODE_SENTINEL
            a_nid[:, bi] = ids
        in_maps.append(
            {
                "x": xf32,
                "src_idx": np.ascontiguousarray(a_src.T),
                "dst_idx": np.ascontiguousarray(a_dst.T),
                "dst_blk": np.ascontiguousarray(a_blk.T),
                "node_ids": a_nid,
                "beta128": beta128,
            }
        )
        if blks:
            core_ranges.append((blks[0][0], blks[-1][1]))
        else:
            core_ranges.append((0, 0))
    return in_maps, core_ranges, NB, T


def _enable_axon_ntff():
    """Install the NTFF profile hook that the stub antenv package lacks."""
    import sys, types
    try:
        import antenv

        if "antenv.axon_hooks" not in sys.modules:
            mod = types.ModuleType("antenv.axon_hooks")
            mod._hook = None
            mod.set_axon_ntff_profile_hook = lambda h: setattr(mod, "_hook", h)
            mod.get_axon_ntff_profile_hook = lambda: mod._hook
            sys.modules["antenv.axon_hooks"] = mod
            antenv.axon_hooks = mod
            from trn_agent_boot.trn_boot import _ntff_profile_via_ctypes

            mod._hook = _ntff_profile_via_ctypes("/opt/axon/libaxon_pjrt.so")
        import concourse.bass_utils as bu

        bu.upload_artifacts = lambda tmpdir: tmpdir
        return True
    except Exception as e:
        print(f"ntff hook install failed: {e}")
        return False


def _run(x, beta, edge_index, trace=False):
    from concourse.bass_utils import run_bass_kernel_spmd

    if trace:
        trace = _enable_axon_ntff()

    N, D = x.shape
    in_maps, core_ranges, NB, T = _prep_inputs(
        x, beta, edge_index, N, D, TPB, NCORES
    )
    nc = _build_graph(N, D, T, NB, TPB)
    res = run_bass_kernel_spmd(
        nc, in_maps, core_ids=list(range(NCORES)), trace=trace
    )
    out = np.zeros((N, D), np.float32)
    for k, (lo, hi) in enumerate(core_ranges):
        if hi > lo:
            out[lo:hi] = res.results[k]["out"][lo:hi]
    return out, res


def kernel(x, beta, edge_index):
    out, _ = _run(
        np.asarray(x), np.asarray(beta), np.asarray(edge_index), trace=False
    )
    return out
